# revision 14
# baseline (speedup 1.0000x reference)
"""GQA kernel for Trainium2, sharded over 8 NeuronCores.

Sharding: data-parallel over batch (2) x tensor-parallel over kv_heads (4).
Core c = b*4 + h computes the full attention output partial
    Y_bh = softmax(causal((Q_b @ Wq_eff_h) @ (K_b @ Wk_h)^T / sqrt(dk))) @ (V_b @ Wv_h) @ Wo_h
and the host sums the 4 head partials per batch (the "all-reduce after Wo").

The GQA group-sum-before-softmax quirk folds into the weights:
    scores_h = sum_g (Q Wq_{g,h}) (K Wk_h)^T = (Q [sum_g Wq_{g,h}]) (K Wk_h)^T
so Wq_eff_h = sum_g Wq[:, (g*KV+h)*dk : ...] and each core runs standard attention.

Device schedule: K, Q and V all stream together through 4 seq-slab stages
(host-packed column-major so each 512KB sub-DMA is one contiguous row-block
and a slab's projection finishes right after its 2MB lands).  Stage j:
  part A: q_j subs -> 16 q-proj matmuls (drip items fill between them)
  part B: k_j + v_j subs -> 32 k/v-proj matmuls alternating PSUM banks
Consecutive matmuls never hit the same PSUM bank back-to-back (that costs a
~2x accumulate stall), and all attention work rides a FIFO "drip" queue
popped between projection matmuls so the in-order PE queue never head-of-line
blocks on the scalar exp latency:
  stage j end pushes: v-transposes(j), scores(j) diag tiles, PV(j) (ot ring),
  rowsum-matmul(j), normalize(j), Y(j) pieces (evict + store on gpsimd queue)
  stage j mid (after qT_j evict) pushes scores(j, non-diag) early.
Row sums: fp16 vector adds of the exp'd S^T tiles + ONE ones-matmul per
chunk.  After stage 3 only chunk-3 attention + Y remain; chunk 3 is split
into two 256-query halves so half A's Y overlaps half B's PV and the tail
after the last PV matmul is only ~half a chunk of Y evicts + stores.
Output stores alternate between the gpsimd and scalar DMA queues (the sync
queue must stay clear: a store there head-of-line blocks input streaming).
Note: the NeuronCore power-throttles (~50% util clamp) for ~35us of the
run (inherited at start + engaged near the end); the schedule minimizes
work in the throttled windows but the tax is ambient.

PSUM: 8 banks = acc ring(3: k/q/v projections) + work ring(3: score/
transpose/rowsum/Y) + ot ring(2: PV accumulators, chunk-pipelined).
"""
import sys
sys.path.insert(0, '/opt/trn_rl_repo')
import math
import numpy as np

import concourse.bass as bass
import concourse.mybir as mybir
import concourse.tile as tile
from concourse import bacc
from concourse import bass_utils
from concourse.masks import make_identity

FP32 = mybir.dt.float32
FP16 = mybir.dt.float16

B, L, D = 2, 2048, 2048
Q_HEADS, KV_HEADS, DK, DV = 16, 4, 128, 128
GROUPS = Q_HEADS // KV_HEADS
P = 128
CH = 512                 # seq slab width (queries and keys)
NJ = L // CH             # 4 slabs
NDC = D // P             # 16 contraction chunks
NSUB = 4                 # sub-DMAs per slab (4 dc each, 512KB)
SCALE = 1.0 / math.sqrt(DK)
EBIAS = -8.0 * math.log(2.0)   # exp output scaled by 2^-8; cancels in softmax
YDT = FP16               # partial-output dtype (host accumulates in fp32)

# et tile offsets, (j, c) j-major causal
ET_OFF = {}
_off = 0
for _j in range(NJ):
    for _c in range(4 * _j + 4):
        ET_OFF[(_j, _c)] = _off
        _off += CH
ET_W = _off              # 40 * 512 fp16 = 40KB/partition


def _build():
    nc = bacc.Bacc(trn_type="TRN2")
    # activations host-packed: row (j*4+s)*128+p, col dcs*512+c holds
    # X[j*512+c, (s*4+dcs)*128+p]; each [128,2048] row-block is one
    # contiguous 512KB sub-slab covering d-chunks [4s, 4s+4) of seq slab j.
    qx_d = nc.dram_tensor("qx", (L, D), FP16, kind="ExternalInput")
    kx_d = nc.dram_tensor("kx", (L, D), FP16, kind="ExternalInput")
    vx_d = nc.dram_tensor("vx", (L, D), FP16, kind="ExternalInput")
    # weights pre-packed on host to the SBUF image: (128, NDC*dk)
    wq_d = nc.dram_tensor("wq", (P, NDC * DK), FP16, kind="ExternalInput")
    wk_d = nc.dram_tensor("wk", (P, NDC * DK), FP16, kind="ExternalInput")
    wv_d = nc.dram_tensor("wv", (P, NDC * DV), FP16, kind="ExternalInput")
    wo_d = nc.dram_tensor("wo", (DV, D), FP16, kind="ExternalInput")
    y_d = nc.dram_tensor("y", (L, D), YDT, kind="ExternalOutput")

    with tile.TileContext(nc) as tc:
        with (
            tc.tile_pool(name="const", bufs=1) as const,
            tc.tile_pool(name="wpool", bufs=1) as wpool,
            tc.tile_pool(name="xs", bufs=22) as xs,
            tc.tile_pool(name="proj", bufs=1) as proj,
            tc.tile_pool(name="ev", bufs=3) as ev_pool,
            tc.tile_pool(name="ps", bufs=3, space="PSUM") as ps,
        ):
            ident = const.tile([P, P], FP16)
            make_identity(nc, ident[:])
            ones = const.tile([P, P], FP16)
            nc.vector.memset(ones[:], 1.0)
            ebias = const.tile([P, 1], FP32)
            nc.vector.memset(ebias[:], EBIAS)
            # causal mask, built on-device: maskt[p, d*CH+x] = (128d+p <= x)
            maskt = const.tile([P, NJ * CH], FP16)
            nc.gpsimd.memset(maskt[:], 1.0)
            for dd in range(4):
                nc.gpsimd.affine_select(
                    out=maskt[:, dd * CH:(dd + 1) * CH],
                    in_=maskt[:, dd * CH:(dd + 1) * CH],
                    compare_op=mybir.AluOpType.is_ge,
                    fill=0.0, base=-128 * dd,
                    pattern=[[1, CH]], channel_multiplier=-1)

            kT = proj.tile([P, L], FP16, tag="kT")
            qT = proj.tile([P, L], FP16, tag="qT")
            v_nat = proj.tile([P, L], FP16, tag="v_nat")
            oT = proj.tile([P, L], FP16, tag="oT")
            et_all = proj.tile([P, ET_W], FP16, tag="et_all")
            ssum = proj.tile([P, NJ * CH], FP16, tag="ssum")
            rinv_all = proj.tile([P, NJ * CH], FP32, tag="rinv_all")

            wk_sb = wpool.tile([P, NDC * DK], FP16, tag="wk")
            wq_sb = wpool.tile([P, NDC * DK], FP16, tag="wq")
            wv_sb = wpool.tile([P, NDC * DV], FP16, tag="wv")
            wo_sb = wpool.tile([DV, D], FP16, tag="wo")

            # ---- drip queue: attention work interleaved into proj mms ---
            drip = []
            di = [0]

            def pop_drip(n):
                while n > 0 and di[0] < len(drip):
                    drip[di[0]]()
                    di[0] += 1
                    n -= 1

            def st_item(j, c):
                def f():
                    st = ps.tile([P, CH], FP32, tag="work", name="st")
                    nc.tensor.matmul(st[:], kT[:, c * P:(c + 1) * P],
                                     qT[:, j * CH:(j + 1) * CH],
                                     start=True, stop=True)
                    et = et_all[:, ET_OFF[(j, c)]:ET_OFF[(j, c)] + CH]
                    nc.scalar.activation(et, st[:],
                                         mybir.ActivationFunctionType.Exp,
                                         bias=ebias[:], scale=SCALE)
                    d = c - 4 * j
                    if d >= 0:   # diagonal tile: zero out k > q
                        nc.vector.tensor_mul(et, et,
                                             maskt[:, d * CH:(d + 1) * CH])
                    ss = ssum[:, j * CH:(j + 1) * CH]
                    if c == 1:
                        e0 = et_all[:, ET_OFF[(j, 0)]:ET_OFF[(j, 0)] + CH]
                        nc.vector.tensor_add(ss, e0, et)
                    elif c > 1:
                        nc.vector.tensor_add(ss, ss, et)
                return f

            def tp_item(c, vTc, t):
                def f():
                    tp = ps.tile([P, P], FP16, tag="work", name="tp")
                    nc.tensor.transpose(tp[:], vTc[:, t * P:(t + 1) * P],
                                        ident[:])
                    nc.vector.tensor_copy(
                        v_nat[:, c * P:(c + 1) * P], tp[:])
                return f

            ot_slot = {}
            HCH = CH // 2

            def pv_item(j, c):
                def f():
                    if c == 0:
                        ot_slot[j] = ps.tile([P, CH], FP32, tag="ot",
                                             bufs=2, name=f"ot{j}")
                    nc.tensor.matmul(
                        ot_slot[j][:], v_nat[:, c * P:(c + 1) * P],
                        et_all[:, ET_OFF[(j, c)]:ET_OFF[(j, c)] + CH],
                        start=(c == 0), stop=(c == 4 * j + 3))
                return f

            def pv_half_item(j, c, h):
                def f():
                    if c == 0:
                        ot_slot[(j, h)] = ps.tile([P, HCH], FP32, tag="ot",
                                                  bufs=2, name=f"ot{j}{h}")
                    e0 = ET_OFF[(j, c)] + h * HCH
                    nc.tensor.matmul(
                        ot_slot[(j, h)][:], v_nat[:, c * P:(c + 1) * P],
                        et_all[:, e0:e0 + HCH],
                        start=(c == 0), stop=(c == 4 * j + 3))
                return f

            def rrep_item(j):
                def f():
                    rrep = ps.tile([P, CH], FP32, tag="work", name="rrep")
                    nc.tensor.matmul(rrep[:], ones[:],
                                     ssum[:, j * CH:(j + 1) * CH],
                                     start=True, stop=True)
                    nc.vector.reciprocal_approx_fast(
                        rinv_all[:, j * CH:(j + 1) * CH], rrep[:])
                return f

            def norm_item(j, t, half=False):
                def f():
                    lq = j * CH + t * P
                    src_ = (ot_slot[(j, t // 2)][:, (t % 2) * P:(t % 2 + 1) * P]
                            if half else ot_slot[j][:, t * P:(t + 1) * P])
                    nc.vector.tensor_mul(oT[:, lq:lq + P], src_,
                                         rinv_all[:, lq:lq + P])
                return f

            def y_item(j, t, split=False):
                def f():
                    lq0 = j * CH + t * P
                    yev = ev_pool.tile([P, D], YDT, tag="yev", name="yev")
                    for dch in range(D // CH):
                        # last chunk rotates over work+acc rings (acc is free
                        # after the v3 projection) for deeper mm/evict overlap
                        ytag = "acc" if (split and dch % 2 == 1) else "work"
                        yps = ps.tile([P, CH], FP32, tag=ytag, name="yps")
                        nc.tensor.matmul(yps[:], oT[:, lq0:lq0 + P],
                                         wo_sb[:, dch * CH:(dch + 1) * CH],
                                         start=True, stop=True)
                        dst = yev[:, dch * CH:(dch + 1) * CH]
                        if dch % 2 == 0:
                            nc.vector.tensor_copy(dst, yps[:])
                        else:
                            nc.scalar.copy(dst, yps[:])
                    # alternate store queues: 2x dispatch/transfer
                    # parallelism (never sync: it would head-of-line block
                    # the input sub-slab stream)
                    if t % 2 == 0:
                        nc.gpsimd.dma_start(y_d[lq0:lq0 + P, :], yev[:])
                    else:
                        nc.scalar.dma_start(y_d[lq0:lq0 + P, :], yev[:])
                return f

            def push_stage_end(j, vTc):
                if j == NJ - 1:
                    # post phase, chunk 3 split into two 256-query halves:
                    # half A's PV/norm/Y overlap half B's PV chain, so the
                    # tail after the last PV is only half a chunk of Y.
                    pa = [pv_half_item(j, c, 0) for c in range(4 * j + 4)]
                    pb = [pv_half_item(j, c, 1) for c in range(4 * j + 4)]
                    for t in range(4):
                        drip.append(tp_item(4 * j + t, vTc, t))
                        drip.append(st_item(j, 4 * j + t))
                        drip.append(pa[2 * t])
                        drip.append(pa[2 * t + 1])
                    drip.extend(pa[8:])
                    drip.append(rrep_item(j))
                    drip.append(norm_item(j, 0, half=True))
                    drip.append(norm_item(j, 1, half=True))
                    drip.append(pb[0])
                    drip.append(y_item(j, 0, split=True))
                    drip.extend(pb[1:3])
                    drip.append(y_item(j, 1, split=True))
                    drip.extend(pb[3:])
                    drip.append(norm_item(j, 2, half=True))
                    drip.append(norm_item(j, 3, half=True))
                    drip.append(y_item(j, 2, split=True))
                    drip.append(y_item(j, 3, split=True))
                    return
                for t in range(4):
                    drip.append(tp_item(4 * j + t, vTc, t))
                for c in range(4 * j, 4 * j + 4):   # diag: need kT(j)
                    drip.append(st_item(j, c))
                for c in range(4 * j + 4):
                    drip.append(pv_item(j, c))
                drip.append(rrep_item(j))
                for t in range(4):
                    drip.append(norm_item(j, t))
                for t in range(4):
                    drip.append(y_item(j, t, split=(j == NJ - 1)))

            # ---- stage 0: k/q/v 3-stream interleaved ------------------
            nc.scalar.dma_start(wk_sb[:, 0:P], wk_d[:, 0:P])
            nc.scalar.dma_start(wq_sb[:, 0:P], wq_d[:, 0:P])
            nc.scalar.dma_start(wv_sb[:, 0:P], wv_d[:, 0:P])
            nc.scalar.dma_start(wk_sb[:, P:4 * P], wk_d[:, P:4 * P])
            nc.scalar.dma_start(wq_sb[:, P:4 * P], wq_d[:, P:4 * P])
            nc.scalar.dma_start(wv_sb[:, P:4 * P], wv_d[:, P:4 * P])
            nc.scalar.dma_start(wk_sb[:, 4 * P:], wk_d[:, 4 * P:])
            nc.scalar.dma_start(wq_sb[:, 4 * P:], wq_d[:, 4 * P:])
            nc.scalar.dma_start(wv_sb[:, 4 * P:], wv_d[:, 4 * P:])
            nc.scalar.dma_start(wo_sb[:], wo_d[:])
            kacc = ps.tile([P, CH], FP32, tag="acc", name="kacc")
            qacc = ps.tile([P, CH], FP32, tag="acc", name="qacc")
            vacc = ps.tile([P, CH], FP32, tag="acc", name="vacc")
            for s in range(NSUB):
                r0 = s * P
                kxt = xs.tile([P, NSUB * CH], FP16, tag="xt", name="kxt")
                if s == 0:   # split first sub so the first mm starts sooner
                    nc.sync.dma_start(kxt[:, 0:CH], kx_d[r0:r0 + P, 0:CH])
                    nc.sync.dma_start(kxt[:, CH:2 * CH], kx_d[r0:r0 + P, CH:2 * CH])
                    nc.sync.dma_start(kxt[:, 2 * CH:], kx_d[r0:r0 + P, 2 * CH:])
                else:
                    nc.sync.dma_start(kxt[:], kx_d[r0:r0 + P, :])
                qxt = xs.tile([P, NSUB * CH], FP16, tag="xt", name="qxt")
                if s == 0:
                    nc.sync.dma_start(qxt[:, 0:CH], qx_d[r0:r0 + P, 0:CH])
                    nc.sync.dma_start(qxt[:, CH:], qx_d[r0:r0 + P, CH:])
                else:
                    nc.sync.dma_start(qxt[:], qx_d[r0:r0 + P, :])
                vxt = xs.tile([P, NSUB * CH], FP16, tag="xt", name="vxt")
                if s == 0:
                    nc.sync.dma_start(vxt[:, 0:CH], vx_d[r0:r0 + P, 0:CH])
                    nc.sync.dma_start(vxt[:, CH:], vx_d[r0:r0 + P, CH:])
                else:
                    nc.sync.dma_start(vxt[:], vx_d[r0:r0 + P, :])
                for dcs in range(4):
                    dc = s * 4 + dcs
                    for acc, w_sb, xt in ((kacc, wk_sb, kxt),
                                          (qacc, wq_sb, qxt),
                                          (vacc, wv_sb, vxt)):
                        nc.tensor.matmul(
                            acc[:], w_sb[:, dc * P:(dc + 1) * P],
                            xt[:, dcs * CH:(dcs + 1) * CH],
                            start=(dc == 0), stop=(dc == NDC - 1))
            nc.vector.tensor_copy(kT[:, 0:CH], kacc[:])
            nc.scalar.copy(qT[:, 0:CH], qacc[:])
            vTc = proj.tile([P, CH], FP16, tag="vTc", bufs=2, name="vTc")
            nc.vector.tensor_copy(vTc[:], vacc[:])
            push_stage_end(0, vTc)

            # ---- stages 1..3: [q subs + drip] then [k/v subs + drip] ---
            for j in range(1, NJ):
                qacc = ps.tile([P, CH], FP32, tag="acc", name="qacc")
                for s in range(NSUB):
                    r0 = (j * NSUB + s) * P
                    qxt = xs.tile([P, NSUB * CH], FP16, tag="xt", name="qxt")
                    nc.sync.dma_start(qxt[:], qx_d[r0:r0 + P, :])
                    for dcs in range(4):
                        dc = s * 4 + dcs
                        # pops lead the matmul so drip work can fill a
                        # q-data stall (the PE queue is in-order)
                        pop_drip(2)
                        nc.tensor.matmul(
                            qacc[:], wq_sb[:, dc * P:(dc + 1) * P],
                            qxt[:, dcs * CH:(dcs + 1) * CH],
                            start=(dc == 0), stop=(dc == NDC - 1))
                nc.scalar.copy(qT[:, j * CH:(j + 1) * CH], qacc[:])
                for c in range(4 * j):      # non-diag tiles: kT(<j) ready
                    drip.append(st_item(j, c))
                kacc = ps.tile([P, CH], FP32, tag="acc", name="kacc")
                vacc = ps.tile([P, CH], FP32, tag="acc", name="vacc")
                for s in range(NSUB):
                    r0 = (j * NSUB + s) * P
                    kxt = xs.tile([P, NSUB * CH], FP16, tag="xt", name="kxt")
                    nc.sync.dma_start(kxt[:], kx_d[r0:r0 + P, :])
                    vxt = xs.tile([P, NSUB * CH], FP16, tag="xt", name="vxt")
                    nc.sync.dma_start(vxt[:], vx_d[r0:r0 + P, :])
                    for dcs in range(4):
                        dc = s * 4 + dcs
                        pop_drip(1)
                        for acc, w_sb, xt in ((kacc, wk_sb, kxt),
                                              (vacc, wv_sb, vxt)):
                            nc.tensor.matmul(
                                acc[:], w_sb[:, dc * P:(dc + 1) * P],
                                xt[:, dcs * CH:(dcs + 1) * CH],
                                start=(dc == 0), stop=(dc == NDC - 1))
                nc.vector.tensor_copy(kT[:, j * CH:(j + 1) * CH], kacc[:])
                vTc = proj.tile([P, CH], FP16, tag="vTc", bufs=2, name="vTc")
                nc.vector.tensor_copy(vTc[:], vacc[:])
                push_stage_end(j, vTc)

            # ---- post phase: chunk-3 attention + Y --------------------
            pop_drip(len(drip))
            assert di[0] == len(drip)
    nc.compile()
    return nc


_NC = None


def _get_nc():
    global _NC
    if _NC is None:
        _NC = _build()
    return _NC


def _pack_w(w):
    """(D, dk) fp32 -> SBUF image (128, NDC*dk): out[p, dc*dk+m] = w[dc*128+p, m]"""
    return np.ascontiguousarray(
        w.reshape(-1, P, w.shape[-1]).transpose(1, 0, 2).reshape(P, -1)).astype(np.float16)


def _pack_act(x):
    """(L, D) fp32 -> packed fp16 (L, D): row (j*4+s)*128+p, col dcs*512+c
    holds x[j*512+c, (s*4+dcs)*128+p]."""
    xt = np.ascontiguousarray(np.asarray(x, np.float32).T)   # (D, L)
    a = xt.reshape(NSUB, 4, P, NJ, CH)        # [s, dcs, p, j, c]
    a = a.transpose(3, 0, 2, 1, 4)            # [j, s, p, dcs, c]
    return np.ascontiguousarray(a.reshape(L, D)).astype(np.float16)


def _make_in_maps(Q, K, V, Wq, Wk, Wv, Wo):
    f16 = np.float16
    # fold GQA group sum into Wq: head = g*KV_HEADS + h
    Wq_eff = np.asarray(Wq, np.float32).reshape(D, GROUPS, KV_HEADS, DK).sum(axis=1)
    acts = {}
    for b in range(B):
        acts[b] = {
            "qx": _pack_act(Q[b]),
            "kx": _pack_act(K[b]),
            "vx": _pack_act(V[b]),
        }
    Wk32, Wv32 = np.asarray(Wk, np.float32), np.asarray(Wv, np.float32)
    Wo32 = np.asarray(Wo, np.float32)
    in_maps = []
    for c in range(8):
        b, h = divmod(c, KV_HEADS)
        in_maps.append({
            **acts[b],
            "wq": _pack_w(Wq_eff[:, h, :]),
            "wk": _pack_w(Wk32[:, h * DK:(h + 1) * DK]),
            "wv": _pack_w(Wv32[:, h * DV:(h + 1) * DV]),
            "wo": Wo32[h * DV:(h + 1) * DV, :].astype(f16),
        })
    return in_maps


def _gather(results):
    Y = np.zeros((B, L, D), np.float32)
    for c in range(8):
        Y[c // KV_HEADS] += results[c]["y"].astype(np.float32)
    return Y


def kernel(Q, K, V, Wq, Wk, Wv, Wo):
    nc = _get_nc()
    in_maps = _make_in_maps(Q, K, V, Wq, Wk, Wv, Wo)
    res = bass_utils.run_bass_kernel_spmd(nc, in_maps, core_ids=list(range(8)))
    return _gather(res.results)


def _install_ntff_hook():
    """The agent image's antenv lacks axon_hooks; synthesize it so
    trace=True can reach the NTFF profiler in libaxon_pjrt.so."""
    import types
    import antenv
    if hasattr(antenv, "axon_hooks"):
        return
    mod = types.ModuleType("antenv.axon_hooks")
    _h = [None]
    mod.set_axon_ntff_profile_hook = lambda h: _h.__setitem__(0, h)
    mod.get_axon_ntff_profile_hook = lambda: _h[0]
    sys.modules["antenv.axon_hooks"] = mod
    antenv.axon_hooks = mod
    from trn_agent_boot.trn_boot import _ntff_profile_via_ctypes
    mod.set_axon_ntff_profile_hook(_ntff_profile_via_ctypes("/opt/axon/libaxon_pjrt.so"))


def kernel_traced(Q, K, V, Wq, Wk, Wv, Wo):
    """Like kernel() but profiles; returns (output, BassKernelResults)."""
    _install_ntff_hook()
    nc = _get_nc()
    in_maps = _make_in_maps(Q, K, V, Wq, Wk, Wv, Wo)
    res = bass_utils.run_bass_kernel_spmd(nc, in_maps, core_ids=list(range(8)),
                                          trace=True)
    return _gather(res.results), res
